# revision 20
# baseline (speedup 1.0000x reference)
"""AttentionCommModule TRN2 kernel: 8-core data-parallel single-query MHA.

Sharding: batch B=32768 split across 8 NeuronCores (4096 rows each); all
weights replicated, no collectives. Inputs are host-packed to bf16 in a
slab-chunk-major, feature-transposed layout [16, 128, bs] so each core
loads activation tiles straight into [k, b] SBUF layout with plain
contiguous DMAs (no on-chip activation transposes at all).

Per 128-row tile on device (batch-major layout, b on partitions):
  TensorE : Q/K/V projections, lhsT = transposed activation chunk
            (stationary), rhs = packed weights, f32 PSUM accumulation;
            plus the 128x128 transpose of `weighted` and the out-proj.
  ScalarE : PSUM -> SBUF copies (cast to bf16), exp().
  VectorE : QK dot-products + halving-tree d-reduction, softmax
            (batched per 4 tiles), attn*V + n-reduction.
The emission is software-pipelined 2 macro-tiles deep, interleaved at
tile granularity, so no engine's in-order stream blocks on the
cross-engine attention chain at macro boundaries.
"""

import numpy as np
import ml_dtypes
from contextlib import ExitStack

import concourse.bass as bass
import concourse.tile as tile
from concourse import bacc, mybir
from concourse.bass_utils import run_bass_kernel_spmd

N_CORES = 8
B_FULL = 32768
INPUT_DIM = 256
COMM = 128
NH = 4
HD = 32
N_MSGS = 7
NS = 8          # slabs = num_agents (self + 7 messages)
TILE = 128      # rows per attention tile
MACRO = 512     # rows per DMA macro-tile

BF = mybir.dt.bfloat16
F32 = mybir.dt.float32
INV_SQRT_HD = 1.0 / float(np.sqrt(HD))
PSUM_OUT_DMA = False  # PSUM is not DMA-addressable on this stack

_compiled = {}


def _build(bs: int, has_bias: bool):
    """Build + compile the per-core Bass program for a bs-row shard."""
    assert bs % MACRO == 0
    nc = bacc.Bacc(
        "TRN2",
        target_bir_lowering=False,
        debug=False,
        enable_asserts=False,
        num_devices=N_CORES,
    )
    # ktpack[c, k, b]: slab-chunk c, feature k on what becomes the SBUF
    # partition dim, batch contiguous — host pre-transposed.
    xpack = nc.dram_tensor("xpack", [2 * NS, 128, bs], BF, kind="ExternalInput").ap()
    wpack = nc.dram_tensor("wpack", [128, 2, 384], BF, kind="ExternalInput").ap()
    wod = nc.dram_tensor("wo", [128, 128], BF, kind="ExternalInput").ap()
    identd = nc.dram_tensor("ident", [128, 128], BF, kind="ExternalInput").ap()
    if has_bias:
        bkvd = nc.dram_tensor("bkv", [1, 2176], F32, kind="ExternalInput").ap()
        bod = nc.dram_tensor("bo", [1, 128], F32, kind="ExternalInput").ap()
    outd = nc.dram_tensor("out", [bs, 128], F32, kind="ExternalOutput").ap()

    with tile.TileContext(nc) as tc, ExitStack() as ctx:
        consts = ctx.enter_context(tc.tile_pool(name="consts", bufs=1))
        xtp = ctx.enter_context(tc.tile_pool(name="xtp", bufs=3))
        kvp = ctx.enter_context(tc.tile_pool(name="kvp", bufs=4))
        work = ctx.enter_context(tc.tile_pool(name="work", bufs=2))
        sm = ctx.enter_context(tc.tile_pool(name="sm", bufs=2))
        osb = ctx.enter_context(tc.tile_pool(name="osb", bufs=2))
        psum = ctx.enter_context(tc.tile_pool(name="psum", bufs=6, space="PSUM"))
        psO = ctx.enter_context(tc.tile_pool(name="psO", bufs=1, space="PSUM"))

        w_sb = consts.tile([128, 768], BF)
        nc.gpsimd.dma_start(w_sb[:].rearrange("p (c n) -> p c n", c=2), wpack[:, :, :])
        wo_sb = consts.tile([128, 128], BF)
        nc.gpsimd.dma_start(wo_sb[:], wod[:, :])
        ident_sb = consts.tile([128, 128], BF)
        nc.gpsimd.dma_start(ident_sb[:], identd[:, :])
        if has_bias:
            bkv_sb = consts.tile([1, 2176], F32)
            nc.gpsimd.dma_start(bkv_sb[:], bkvd[:, :])
            bo_sb = consts.tile([1, 128], F32)
            nc.gpsimd.dma_start(bo_sb[:], bod[:, :])

        n_macro = bs // MACRO
        # Software pipeline, 2 macros deep, interleaved at tile granularity:
        # each engine's in-order stream alternates [pass2-tile(m-2, j),
        # pass1-tile(m, j)] so no pass-2 dependency (DVE tail -> xbar ->
        # out-proj matmul) ever blocks the next macro's projection work.
        def emit_load(m):
            # ---- input load: ONE coalesced DMA on the idle sync engine ----
            # (a DMA trigger costs ~667ns of ScalarE time vs ~600ns on the
            # otherwise-idle sync queue; 16-to-1 coalescing also drops 15
            # trigger instructions per macro)
            xt = xtp.tile([128, 16 * MACRO], BF, name=f"xt{m}", tag="xt")
            nc.sync.dma_start(
                xt[:].rearrange("p (c n) -> p c n", c=16),
                xpack[:, :, m * MACRO:(m + 1) * MACRO].transpose([1, 0, 2]),
            )
            return xt

        KV = 2176  # per-tile [Q | K0 V0 | ... | K7 V7] span in kv4

        def emit_mm_tile(m, j, xt, kv4):
            # kv4 SBUF layout: [Q0..Q3 | KV_0 | KV_1 | KV_2 | KV_3] with
            # KV_t = [K0 V0 ... K7 V7] (2048). Q de-interleaved so the p2
            # views have nestable (t, s) dims -> single 3-dim DVE op.
            psQ = psum.tile([128, 128], F32, tag="ps", name=f"psQ{m}_{j}")
            ps0 = psum.tile([128, 256], F32, tag="ps", name=f"psz{m}_{j}")
            for ch in (0, 1):
                lhsT = xt[:, ch * MACRO + j * TILE: ch * MACRO + j * TILE + 128]
                nc.tensor.matmul(psQ[:, :], lhsT=lhsT,
                                 rhs=w_sb[:, ch * 384:ch * 384 + 128],
                                 start=(ch == 0), stop=(ch == 1))
                nc.tensor.matmul(ps0[:, :], lhsT=lhsT,
                                 rhs=w_sb[:, ch * 384 + 128:(ch + 1) * 384],
                                 start=(ch == 0), stop=(ch == 1))
            kvtiles = [ps0]
            for pi, pair in enumerate(((1, 2), (3, 4), (5, 6), (7,))):
                width = 256 * len(pair)
                ps = psum.tile([128, width], F32, tag="ps", name=f"ps{pi}_{m}_{j}")
                for si, s in enumerate(pair):
                    for ch in (0, 1):
                        nc.tensor.matmul(
                            ps[:, si * 256:(si + 1) * 256],
                            lhsT=xt[:, (2 * s + ch) * MACRO + j * TILE:
                                    (2 * s + ch) * MACRO + j * TILE + 128],
                            rhs=w_sb[:, ch * 384 + 128:(ch + 1) * 384],
                            start=(ch == 0),
                            stop=(ch == 1),
                        )
                kvtiles.append(ps)

            # PSUM -> SBUF copies: ScalarE (Q + big tiles) + DVE (last one)
            nc.scalar.copy(kv4[:, j * 128:(j + 1) * 128], psQ[:, :])
            off = 512 + j * 2048
            for pi, ps in enumerate(kvtiles):
                w = ps.shape[1]
                if pi == len(kvtiles) - 1:
                    nc.vector.tensor_copy(kv4[:, off:off + w], ps[:, :])
                else:
                    nc.scalar.copy(kv4[:, off:off + w], ps[:, :])
                off += w
            if has_bias:
                nc.vector.tensor_add(
                    kv4[:, j * 128:(j + 1) * 128],
                    kv4[:, j * 128:(j + 1) * 128],
                    bkv_sb[:, 0:128].partition_broadcast(128),
                )
                nc.vector.tensor_add(
                    kv4[:, 512 + j * 2048:512 + (j + 1) * 2048],
                    kv4[:, 512 + j * 2048:512 + (j + 1) * 2048],
                    bkv_sb[:, 128:KV].partition_broadcast(128),
                )

        G = 4 * NS * NH

        def emit_chainA(m, st):
            """p1 = Q*K, d-reduction, exp — consumed one round later."""
            kv4 = st["kv4"]
            qb = (
                kv4[:, 0:512]
                .rearrange("p (t c) -> p t c", t=4)
                .unsqueeze(2)
                .broadcast_to([128, 4, NS, 128])
            )
            kk = kv4[:, 512:512 + 4 * 2048].rearrange(
                "p (t s kv c) -> p t s kv c", t=4, s=NS, kv=2
            )[:, :, :, 0, :]
            p1 = work.tile([128, 4096], BF, tag="p1", name=f"p1_{m}")
            nc.vector.tensor_mul(
                p1[:].rearrange("p (t s c) -> p t s c", t=4, s=NS),
                qb, kk,
            )
            # d-reduction tree (32 -> 1 per g=(t,s,h)): big level on DVE
            # (2x bf16), tail on the idle GPSIMD
            scores4 = sm.tile([128, 128], F32, tag="scores4", name=f"sc4_{m}")
            cur, cd = p1, HD
            for r in range(4):
                nxt = work.tile(
                    [128, G * cd // 2], BF, tag=f"t{r}", name=f"t{r}_{m}",
                )
                v = cur[:].rearrange("p (g e d) -> p g e d", g=G, e=2)
                eng = nc.vector if r == 0 else nc.gpsimd
                eng.tensor_add(
                    nxt[:].rearrange("p (g d) -> p g d", g=G),
                    v[:, :, 0, :],
                    v[:, :, 1, :],
                )
                cur, cd = nxt, cd // 2
            v = cur[:].rearrange("p (g e) -> p g e", g=G, e=2)
            nc.gpsimd.tensor_add(
                scores4[:].rearrange("p (g o) -> p g o", g=G),
                v[:, :, 0:1],
                v[:, :, 1:2],
            )
            # exp (max-subtraction skipped: scores/sqrt(d) are O(1) for this
            # problem's scale, exp cannot overflow)
            e4 = sm.tile([128, 128], BF, tag="e4", name=f"e4_{m}")
            nc.scalar.activation(
                e4[:], scores4[:], mybir.ActivationFunctionType.Exp,
                scale=INV_SQRT_HD,
            )
            st["e4"] = e4

        def emit_chainB(m, st):
            """softmax normalization + attn*V + s-reduction -> wtd4."""
            kv4 = st["kv4"]
            e4 = st["e4"]
            s4 = sm.tile([128, 16], F32, tag="s4", name=f"s4_{m}")
            e_tsh = e4[:].rearrange("p (t s h) -> p t s h", t=4, s=NS)
            nc.vector.reduce_sum(s4[:].rearrange("p (t h) -> p t h", t=4),
                                 e_tsh.transpose([0, 1, 3, 2]),
                                 axis=mybir.AxisListType.X)
            r4 = sm.tile([128, 16], F32, tag="r4", name=f"r4_{m}")
            nc.vector.reciprocal(r4[:], s4[:])
            a4 = sm.tile([128, 128], BF, tag="a4", name=f"a4_{m}")
            r4b = (
                r4[:]
                .rearrange("p (t h) -> p t h", t=4)
                .unsqueeze(2)
                .broadcast_to([128, 4, NS, NH])
            )
            nc.vector.tensor_mul(
                a4[:].rearrange("p (t s h) -> p t s h", t=4, s=NS), e_tsh, r4b
            )
            # p2 = attn * V in ONE DVE op: (t, s) nests into a single dim
            # since Q is de-interleaved. V columns are d-major so the attn
            # broadcast is stride-1 innermost (DVE 2x).
            p2 = work.tile([128, 4096], BF, tag="p2", name=f"p2_{m}")
            ab = (
                a4[:]
                .rearrange("p (ts h) -> p ts h", ts=32)
                .unsqueeze(2)
                .broadcast_to([128, 32, HD, NH])
            )
            vv = kv4[:, 512:512 + 4 * 2048].rearrange(
                "p (ts kv d h) -> p ts kv d h", ts=32, kv=2, d=HD
            )[:, :, 1, :, :]
            nc.vector.tensor_mul(
                p2[:].rearrange("p (ts d h) -> p ts d h", ts=32, d=HD),
                ab, vv,
            )
            # s-reduction tree (fold the middle s dim of (t, s, c)): big
            # levels on DVE, small tail on GPSIMD
            cur, cn = p2, NS
            for r in range(3):
                nxt = work.tile(
                    [128, cn * 256], BF, tag=f"v{r}", name=f"v{r}_{m}"
                )
                v = cur[:].rearrange("p (t s c) -> p t s c", t=4, s=cn)
                eng = nc.vector if r == 0 else nc.gpsimd
                eng.tensor_add(
                    nxt[:].rearrange("p (t s c) -> p t s c", t=4, s=cn // 2),
                    v[:, :, 0:cn // 2, :],
                    v[:, :, cn // 2:cn, :],
                )
                cur, cn = nxt, cn // 2
            st["wtd4"] = cur  # [128, 512] bf16, (t, c) with c=(d,h) d-major

        def emit_transposes(m, st):
            # PE reaches these after the round's MMs; wtd4 (chainB, emitted
            # earlier this round on DVE) is ready by then. The wtdT4 SBUF
            # copy is DVE's last op of the round; its consumer (out-proj) is
            # scheduled at the TOP of the next PE round.
            wtd4 = st["wtd4"]
            ptp4 = psO.tile([128, 512], BF, tag="ptp4", name=f"ptp4_{m}")
            for t in range(4):
                nc.tensor.transpose(ptp4[:, t * 128:(t + 1) * 128],
                                    wtd4[:, t * 128:(t + 1) * 128],
                                    ident_sb[:])
            wtdT4 = work.tile([128, 512], BF, tag="wtdT4", name=f"wtdT4_{m}")
            nc.vector.tensor_copy(wtdT4[:], ptp4[:])
            st["wtdT4"] = wtdT4

        def emit_outproj(m, st):
            wtdT4 = st["wtdT4"]
            po4 = psO.tile([128, 512], F32, tag="po4", name=f"po4_{m}")
            for t in range(4):
                nc.tensor.matmul(po4[:, t * 128:(t + 1) * 128],
                                 lhsT=wtdT4[:, t * 128:(t + 1) * 128],
                                 rhs=wo_sb[:], start=True, stop=True)
            out_sb = osb.tile([128, 4 * TILE], F32, tag="out_sb",
                              name=f"osb{m}")
            nc.scalar.copy(out_sb[:], po4[:])
            if has_bias:
                nc.vector.tensor_add(
                    out_sb[:].rearrange("p (t j) -> p t j", t=4),
                    out_sb[:].rearrange("p (t j) -> p t j", t=4),
                    bo_sb[:, :].partition_broadcast(128).unsqueeze(1)
                    .broadcast_to([128, 4, 128]),
                )
            nc.sync.dma_start(
                outd[m * MACRO:(m + 1) * MACRO, :].rearrange(
                    "(t p) j -> p t j", t=4
                ),
                out_sb[:].rearrange("p (t j) -> p t j", t=4),
            )

        # Software pipeline at macro granularity. Emission order per round r:
        #   1. out-proj(r-2) + output copy/DMA   (inputs one round stale)
        #   2. loads + projection MMs + PSUM->SBUF copies (r)
        #   3. chainB(r-1): softmax tail + attn*V  (e4 one round stale)
        #   4. chainA(r): QK + d-reduce + exp     (kv4 copies same round)
        #   5. transposes(r-1)                    (wtd4 from step 3)
        # PE's stream [outproj | MMs | transposes] never waits on same-round
        # work; DVE's only same-round dependency (p1 on the copies) is
        # buffered by chainB work in front of it.
        state = {}
        for r in range(n_macro + 2):
            if 0 <= r - 2 < n_macro:
                emit_outproj(r - 2, state[r - 2])
                del state[r - 2]
            if r < n_macro:
                st = state[r] = {
                    "xt": emit_load(r),
                    "kv4": kvp.tile([128, 4 * KV], BF, tag="kv4",
                                    name=f"kv4_{r}"),
                }
                for j in range(4):
                    emit_mm_tile(r, j, st["xt"], st["kv4"])
            if 0 <= r - 1 < n_macro:
                emit_chainB(r - 1, state[r - 1])
            if r < n_macro:
                emit_chainA(r, state[r])
            if 0 <= r - 1 < n_macro:
                emit_transposes(r - 1, state[r - 1])

    nc.compile()
    return nc


def _get_compiled(bs: int, has_bias: bool):
    key = (bs, has_bias)
    if key not in _compiled:
        _compiled[key] = _build(bs, has_bias)
    return _compiled[key]


def _pack_inputs(agent_obs, messages, Wq, bq, Wk, bk, Wv, bv, Wo, bo):
    """Host-side packing (per full batch): returns dict of device arrays."""
    bf16 = ml_dtypes.bfloat16
    b = agent_obs.shape[0]
    allm = np.concatenate([agent_obs[:, None, :], messages], axis=1)  # [b, 8, 256]
    # slab-chunk-major, feature-transposed: xpack[2s+ch, k, b]
    xpack = np.ascontiguousarray(
        allm.reshape(b, NS, 2, 128).transpose(1, 2, 3, 0).reshape(16, 128, b)
    ).astype(bf16)

    # V (and Wo rows) in d-major column order c' = d*NH + h so the DVE
    # attn broadcast is stride-1 innermost.
    perm = (np.arange(128).reshape(NH, HD).T).reshape(-1)  # c' -> h*HD+d
    WvTp = Wv.T[:, perm]
    wcat = np.concatenate([Wq.T, Wk.T, WvTp], axis=1)  # [256, 384]
    wpack = np.ascontiguousarray(
        wcat.reshape(2, 128, 384).transpose(1, 0, 2)
    ).astype(bf16)  # [128, 2, 384]
    wo = np.ascontiguousarray(Wo.T[perm, :]).astype(bf16)  # [128, 128]

    has_bias = bool(
        np.any(bq != 0) or np.any(bk != 0) or np.any(bv != 0) or np.any(bo != 0)
    )
    extra = {"ident": np.eye(128, dtype=bf16)}
    if has_bias:
        # PSUM layout [Q | K0 V0 | ... | K7 V7]
        bkv = np.zeros((1, 2176), np.float32)
        bkv[0, 0:128] = bq
        for s in range(NS):
            bkv[0, 128 + s * 256:128 + s * 256 + 128] = bk
            bkv[0, 256 + s * 256:256 + s * 256 + 128] = bv[perm]
        extra["bkv"] = bkv
        extra["bo"] = bo.reshape(1, 128).astype(np.float32)
    return xpack, wpack, wo, extra, has_bias


def kernel(agent_obs, messages, Wq, bq, Wk, bk, Wv, bv, Wo, bo):
    b = agent_obs.shape[0]
    assert b % N_CORES == 0
    bs = b // N_CORES

    xpack, wpack, wo, extra, has_bias = _pack_inputs(
        np.asarray(agent_obs, np.float32), np.asarray(messages, np.float32),
        np.asarray(Wq, np.float32), np.asarray(bq, np.float32),
        np.asarray(Wk, np.float32), np.asarray(bk, np.float32),
        np.asarray(Wv, np.float32), np.asarray(bv, np.float32),
        np.asarray(Wo, np.float32), np.asarray(bo, np.float32),
    )
    nc = _get_compiled(bs, has_bias)

    in_maps = []
    for c in range(N_CORES):
        m = {
            "xpack": np.ascontiguousarray(xpack[:, :, c * bs:(c + 1) * bs]),
            "wpack": wpack,
            "wo": wo,
        }
        m.update(extra)
        in_maps.append(m)

    res = run_bass_kernel_spmd(nc, in_maps, core_ids=list(range(N_CORES)))
    out = np.concatenate([r["out"] for r in res.results], axis=0)
    return out.astype(np.float32)

